# revision 9
# baseline (speedup 1.0000x reference)
"""ClusterGCN layer on 8 TRN2 NeuronCores.

Math: for each cluster c (only intra-cluster edges matter),
    Y_c = B_c @ (X_c @ W) + b
    B_c[d, s] = dis[d] * At_c[s, d] * dis[s]
    At_c[s, d] = #edges(s->d in c) + [s == d]     (self-loop: dis^2 = 1/deg)
with dis = rsqrt(deg), deg = intra in-degree + 1. Clusters with no intra
edge pass X through unchanged (patched on host).

Host pre-scales X by dis[s] so the device pipeline is pure matmul + cast:
  step1: xws = (X*dis) @ W       -- nodes on partitions, fp16 PE, fp32 PSUM.
         Two s-chunks accumulate into one 2KB PSUM bank (a single
         accumulation group; the first matmul's start zeroes the bank).
  step2: Z^T[f, d] = sum_s xws[s, f] * At[s, d];  Y = dis[d] * Z[d] + b on
         host. At ships as fp8e4m3 edge counts (small integers are exact).

DoubleRow mode (KDR=1): step-2 runs fp8xfp8 MatmulPerfMode.DoubleRow,
contracting two 128-s-chunks per instruction. xws is split hi/lo:
  hi = fp8(xws) on the Scalar engine, lo = fp8(xws - hi) on DVE/Pool,
  Z = sum (hi + lo) * At  -- exact to ~1e-3 (residual-of-residual).

All cluster loads are issued upfront on the Sync queue (whole working set
is ~6KB/partition/cluster, far under SBUF); stores go on the Pool queue.
"""

import os

import numpy as np

N_CORES = 8
N_CLUSTERS = 100
P = 128

_DR = os.environ.get("KDR", "0") == "1"

_prog_cache: dict = {}


def _build_program(cpc: int, cap: int, in_c: int, f_out: int, a_fp8: bool,
                   dr: bool):
    """Build + compile the per-core Bass program.

    cpc: clusters per core; cap: padded cluster size (multiple of 256).
    a_fp8: adjacency ships as fp8e4m3 counts (else fp16 counts).
    dr: fp8 DoubleRow step-2 with hi/lo split (requires a_fp8).
    """
    import concourse.mybir as mybir
    import concourse.tile as tile
    from concourse import bacc

    key = (cpc, cap, in_c, f_out, a_fp8, dr)
    if key in _prog_cache:
        return _prog_cache[key]

    kc = in_c // P           # contraction chunks for X @ W
    sch = cap // P           # s chunks per cluster
    hh = sch // 2            # bank-halves (s-chunk pairs) per cluster
    fc = f_out // P          # f chunks (step-2 output partitions)
    f32 = mybir.dt.float32
    fp16 = mybir.dt.float16
    fp8 = mybir.dt.float8e4
    a_dt = fp8 if a_fp8 else fp16
    Copy = mybir.ActivationFunctionType.Copy
    mul_op = mybir.AluOpType.mult
    sub_op = mybir.AluOpType.subtract
    DR = mybir.MatmulPerfMode.DoubleRow

    nc = bacc.Bacc("TRN2", target_bir_lowering=False, debug=False,
                   num_devices=N_CORES)

    XT = nc.dram_tensor("XT", [in_c, cpc * cap], fp16, kind="ExternalInput")
    Wt = nc.dram_tensor("Wt", [in_c, f_out], fp16, kind="ExternalInput")
    AT = nc.dram_tensor("AT", [cpc, P, sch, cap], a_dt, kind="ExternalInput")
    YT = nc.dram_tensor("YT", [cpc, f_out, cap], fp16, kind="ExternalOutput")

    XTr = XT.rearrange("(k p) n -> p k n", p=P)

    YTr = YT.rearrange("c (f p) d -> c p f d", p=P)

    with tile.TileContext(nc) as tc:
        with (
            tc.tile_pool(name="w", bufs=kc) as w_pool,
            tc.tile_pool(name="xt", bufs=cpc + 1) as xt_pool,
            tc.tile_pool(name="at", bufs=cpc) as at_pool,
            tc.tile_pool(name="xw", bufs=3 * hh) as xw_pool,
            tc.tile_pool(name="lo", bufs=3 * hh) as lo_pool,
            tc.tile_pool(name="out", bufs=6) as out_pool,
            tc.tile_pool(name="ps1", bufs=4, space="PSUM") as ps1_pool,
            tc.tile_pool(name="ps2", bufs=4, space="PSUM") as ps2_pool,
        ):
            # first-cluster loads fan out across the idle engine queues AND
            # use separate tiles per half (tile deps are buffer-granular, so
            # a shared tile would make the first matmul wait on every DMA
            # writing it); the rest stream on Sync as whole-cluster tiles.
            Wr = Wt.rearrange("(k p) f -> p k f", p=P)
            wts = [w_pool.tile([P, f_out], fp16) for _ in range(kc)]
            x0h = [xt_pool.tile([P, kc, 2 * P], fp16) for _ in range(hh)]
            at0 = at_pool.tile([P, sch, cap], a_dt)
            nc.scalar.dma_start(wts[0][:], Wr[:, 0])
            nc.gpsimd.dma_start(x0h[0][:], XTr[:, :, :2 * P])
            nc.scalar.dma_start(wts[1][:], Wr[:, 1])
            nc.gpsimd.dma_start(x0h[1][:], XTr[:, :, 2 * P:cap])
            nc.gpsimd.dma_start(at0[:], AT[0])

            xts, ats = [x0h], [at0]
            for c in range(1, cpc):
                xt = xt_pool.tile([P, kc, cap], fp16)
                nc.sync.dma_start(xt[:], XTr[:, :, c * cap:(c + 1) * cap])
                at = at_pool.tile([P, sch, cap], a_dt)
                nc.sync.dma_start(at[:], AT[c])
                xts.append(xt)
                ats.append(at)

            def step1(c):
                """xws = (X*dis) @ W; two s-chunks per PSUM bank, one
                accumulation group (first matmul zeroes the bank)."""
                xt = xts[c]
                banks = []
                for h in range(hh):
                    ps = ps1_pool.tile([P, 2, f_out], f32)
                    for j in range(2):
                        s = 2 * h + j
                        for k in range(kc):
                            if c == 0:
                                lhs = xt[h][:, k, j * P:(j + 1) * P]
                            else:
                                lhs = xt[:, k, s * P:(s + 1) * P]
                            nc.tensor.matmul(
                                ps[:, j, :],
                                lhsT=lhs,
                                rhs=wts[k][:],
                                start=(j == 0 and k == 0),
                                stop=(j == 1 and k == kc - 1),
                                skip_group_check=True,
                            )
                    banks.append(ps)
                return banks

            def evac1(banks):
                """PSUM -> SBUF evacuation of step-1, split ACT/DVE."""
                his, los = [], []
                x_dt = fp8 if dr else fp16
                for h, ps in enumerate(banks):
                    hi = xw_pool.tile([P, 2, f_out], x_dt)
                    # DR: both hi on ACT (DVE owns the lo residuals);
                    # non-DR: one per engine
                    if dr or h == 0:
                        nc.scalar.activation(hi[:], ps[:], Copy)
                    else:
                        nc.vector.tensor_copy(hi[:], ps[:])
                    his.append(hi)
                    if dr:
                        lo = lo_pool.tile([P, 2, f_out], fp8)
                        nc.vector.scalar_tensor_tensor(
                            lo[:], ps[:], 1.0, hi[:], mul_op, sub_op)
                        los.append(lo)
                return his, los

            def step2(c, his, los):
                """Z_c^T[f, d] = sum_s xws[s, f] * At[s, d]; z f-halves
                evac'd and stored independently to keep the tail short."""
                at = ats[c]
                for f in range(fc):
                    ps = ps2_pool.tile([P, cap], f32)
                    if dr:
                        mms = [(his, 0), (his, 1), (los, 0), (los, 1)]
                        for i, (tiles, tp) in enumerate(mms):
                            nc.tensor.matmul(
                                ps[:],
                                lhsT=tiles[tp][:, :, f * P:(f + 1) * P],
                                rhs=at[:, 2 * tp:2 * tp + 2, :],
                                start=(i == 0),
                                stop=(i == len(mms) - 1),
                                perf_mode=DR,
                            )
                    else:
                        for s in range(sch):
                            nc.tensor.matmul(
                                ps[:],
                                lhsT=his[s // 2][:, s % 2, f * P:(f + 1) * P],
                                rhs=at[:, s, :],
                                start=(s == 0),
                                stop=(s == sch - 1),
                            )
                    ot = out_pool.tile([P, cap], fp16)
                    if f == 0:
                        nc.scalar.copy(ot[:], ps[:])
                    else:
                        nc.vector.tensor_copy(ot[:], ps[:])
                    nc.gpsimd.dma_start(YTr[c, :, f], ot[:])

            # software pipeline: step1(c+1) runs on the PE while the ACT/DVE
            # engines evacuate cluster c's xw banks, so step2(c) never waits
            banks = step1(0)
            for c in range(cpc):
                nxt = step1(c + 1) if c + 1 < cpc else None
                his, los = evac1(banks)
                step2(c, his, los)
                banks = nxt

    nc.compile()
    _prog_cache[key] = nc
    return nc


def _host_prep(X, W, b, assign, full_ei):
    """Shard + preprocess. Returns (in_maps, a_fp8, gather info)."""
    n, in_c = X.shape
    f_out = W.shape[1]
    src = full_ei[0].astype(np.int64)
    dst = full_ei[1].astype(np.int64)
    a_s = assign[src]
    intra = a_s == assign[dst]
    es, ed = src[intra], dst[intra]

    deg = np.ones(n, np.float32)
    np.add.at(deg, ed, np.float32(1))
    dis = (1.0 / np.sqrt(deg)).astype(np.float32)

    has_edge = np.zeros(N_CLUSTERS, bool)
    has_edge[np.unique(a_s[intra])] = True

    sizes = np.bincount(assign, minlength=N_CLUSTERS)
    cpc = -(-N_CLUSTERS // N_CORES)            # clusters per core
    cap = max(512, int(-(-sizes.max() // 256)) * 256)  # padded cluster size

    starts = np.zeros(N_CLUSTERS + 1, np.int64)
    starts[1:] = np.cumsum(sizes)
    order = np.argsort(assign, kind="stable")
    pos = np.empty(n, np.int64)
    pos[order] = np.arange(n) - starts[assign[order]]

    ctot = cpc * N_CORES
    # At blocks: At[c][s, d] = #edges(s->d) + [s==d]
    At = np.zeros((ctot, cap, cap), np.uint16)
    np.add.at(At, (assign[es], pos[es], pos[ed]), 1)
    At[assign, pos, pos] += 1
    a_fp8 = int(At.max()) <= 15    # integers <= 16 are exact in e4m3

    import ml_dtypes
    a_np = ml_dtypes.float8_e4m3 if a_fp8 else np.float16

    # pre-scale X rows by dis so the device never touches dis
    Xp = np.zeros((ctot, cap, in_c), np.float32)
    Xp[assign, pos] = X * dis[:, None]
    XT_all = np.ascontiguousarray(
        Xp.reshape(ctot * cap, in_c).T).astype(np.float16)

    # [c, s, d] -> [c, p, so, d] so each partition row is one contiguous run
    sch = cap // P
    At_send = np.ascontiguousarray(
        At.astype(a_np).reshape(-1, sch, P, cap).transpose(0, 2, 1, 3))

    nodes = cpc * cap
    in_maps = []
    for i in range(N_CORES):
        in_maps.append({
            "XT": np.ascontiguousarray(XT_all[:, i * nodes:(i + 1) * nodes]),
            "Wt": W.astype(np.float16),
            "AT": At_send[i * cpc:(i + 1) * cpc],
        })
    return in_maps, a_fp8, (cpc, cap, has_edge, pos, dis)


def _run(inputs, trace=False, tmpdir=None):
    from concourse.bass_utils import run_bass_kernel_spmd

    X = np.asarray(inputs["X"], np.float32)
    W = np.asarray(inputs["W"], np.float32)
    b = np.asarray(inputs["b"], np.float32)
    assign = np.asarray(inputs["assign"])
    full_ei = np.asarray(inputs["full_ei"])

    n, in_c = X.shape
    f_out = W.shape[1]
    in_maps, a_fp8, (cpc, cap, has_edge, pos, dis) = _host_prep(
        X, W, b, assign, full_ei)
    nc = _build_program(cpc, cap, in_c, f_out, a_fp8, _DR and a_fp8)

    res = run_bass_kernel_spmd(
        nc, in_maps, core_ids=list(range(N_CORES)),
        trace=trace, tmpdir=tmpdir,
    )
    # YT: [core][cpc, f_out, cap]; row n lives at [core, lc, :, pos]
    YTdev = np.stack([res.results[i]["YT"] for i in range(N_CORES)])
    if YTdev.dtype != np.float32:
        YTdev = YTdev.astype(np.float32)

    c = assign.astype(np.int64)
    core = c // cpc
    lc = c % cpc
    Y = YTdev[core, lc, :, pos]
    Y *= dis[:, None]
    Y += b[None, :].astype(np.float32)
    miss = ~has_edge[c]
    if miss.any():
        Y[miss] = X[miss]
    return Y, res


def kernel(**inputs) -> np.ndarray:
    Y, _ = _run(inputs)
    return Y


# revision 12
# speedup vs baseline: 1.0578x; 1.0578x over previous
"""ClusterGCN layer on 8 TRN2 NeuronCores.

Math: for each cluster c (only intra-cluster edges matter),
    Y_c = B_c @ (X_c @ W) + b
    B_c[d, s] = dis[d] * At_c[s, d] * dis[s]
    At_c[s, d] = #edges(s->d in c) + [s == d]     (self-loop: dis^2 = 1/deg)
with dis = rsqrt(deg), deg = intra in-degree + 1. Clusters with no intra
edge pass X through unchanged (patched on host).

Host pre-scales X by dis[s] so the device pipeline is pure matmul + cast:
  step1: xws = (X*dis) @ W       -- nodes on partitions, fp16 PE, fp32 PSUM.
         Two s-chunks accumulate into one 2KB PSUM bank (a single
         accumulation group; the first matmul's start zeroes the bank).
  step2: Z^T[f, d] = sum_s xws[s, f] * At[s, d];  Y = dis[d] * Z[d] + b on
         host. At ships as fp8e4m3 edge counts (small integers are exact).

DoubleRow mode (KDR=1): step-2 runs fp8xfp8 MatmulPerfMode.DoubleRow,
contracting two 128-s-chunks per instruction. xws is split hi/lo:
  hi = fp8(xws) on the Scalar engine, lo = fp8(xws - hi) on DVE/Pool,
  Z = sum (hi + lo) * At  -- exact to ~1e-3 (residual-of-residual).

All cluster loads are issued upfront on the Sync queue (whole working set
is ~6KB/partition/cluster, far under SBUF); stores go on the Pool queue.
"""

import os

import numpy as np

N_CORES = 8
N_CLUSTERS = 100
P = 128

_DR = os.environ.get("KDR", "0") == "1"

_prog_cache: dict = {}


def _build_program(cpc: int, cap: int, in_c: int, f_out: int, a_fp8: bool,
                   dr: bool):
    """Build + compile the per-core Bass program.

    cpc: clusters per core; cap: padded cluster size (multiple of 256).
    a_fp8: adjacency ships as fp8e4m3 counts (else fp16 counts).
    dr: fp8 DoubleRow step-2 with hi/lo split (requires a_fp8).
    """
    import concourse.mybir as mybir
    import concourse.tile as tile
    from concourse import bacc

    key = (cpc, cap, in_c, f_out, a_fp8, dr)
    if key in _prog_cache:
        return _prog_cache[key]

    kc = in_c // P           # contraction chunks for X @ W
    sch = cap // P           # s chunks per cluster
    hh = sch // 2            # bank-halves (s-chunk pairs) per cluster
    fc = f_out // P          # f chunks (step-2 output partitions)
    f32 = mybir.dt.float32
    fp16 = mybir.dt.float16
    fp8 = mybir.dt.float8e4
    a_dt = fp8 if a_fp8 else fp16
    Copy = mybir.ActivationFunctionType.Copy
    mul_op = mybir.AluOpType.mult
    sub_op = mybir.AluOpType.subtract
    DR = mybir.MatmulPerfMode.DoubleRow

    nc = bacc.Bacc("TRN2", target_bir_lowering=False, debug=False,
                   num_devices=N_CORES)

    XT = nc.dram_tensor("XT", [in_c, cpc * cap], fp16, kind="ExternalInput")
    Wt = nc.dram_tensor("Wt", [in_c, f_out], fp16, kind="ExternalInput")
    AT = nc.dram_tensor("AT", [cpc, P, sch, cap], a_dt, kind="ExternalInput")
    YT = nc.dram_tensor("YT", [cpc, f_out, cap], fp16, kind="ExternalOutput")

    XTr = XT.rearrange("(k p) n -> p k n", p=P)

    YTr = YT.rearrange("c (f p) d -> c p f d", p=P)

    with tile.TileContext(nc) as tc:
        with (
            tc.tile_pool(name="w", bufs=kc) as w_pool,
            tc.tile_pool(name="xt", bufs=cpc + 1) as xt_pool,
            tc.tile_pool(name="at", bufs=cpc) as at_pool,
            tc.tile_pool(name="xw", bufs=3 * hh) as xw_pool,
            tc.tile_pool(name="lo", bufs=3 * hh) as lo_pool,
            tc.tile_pool(name="out", bufs=6) as out_pool,
            tc.tile_pool(name="ps1", bufs=4, space="PSUM") as ps1_pool,
            tc.tile_pool(name="ps2", bufs=4, space="PSUM") as ps2_pool,
        ):
            # first-cluster loads fan out across the idle engine queues AND
            # use separate tiles per half (tile deps are buffer-granular, so
            # a shared tile would make the first matmul wait on every DMA
            # writing it); the rest stream on Sync as whole-cluster tiles.
            Wr = Wt.rearrange("(k p) f -> p k f", p=P)
            wts = [w_pool.tile([P, f_out], fp16, name=f"wt{k}")
                   for k in range(kc)]
            x0h = [xt_pool.tile([P, kc, 2 * P], fp16, name=f"x0h{h}")
                   for h in range(hh)]
            at0 = at_pool.tile([P, sch, cap], a_dt)
            nc.scalar.dma_start(wts[0][:], Wr[:, 0])
            nc.gpsimd.dma_start(x0h[0][:], XTr[:, :, :2 * P])
            nc.scalar.dma_start(wts[1][:], Wr[:, 1])
            nc.gpsimd.dma_start(x0h[1][:], XTr[:, :, 2 * P:cap])
            nc.sync.dma_start(at0[:], AT[0])

            xts, ats = [x0h], [at0]
            for c in range(1, cpc):
                xt = xt_pool.tile([P, kc, cap], fp16)
                nc.sync.dma_start(xt[:], XTr[:, :, c * cap:(c + 1) * cap])
                at = at_pool.tile([P, sch, cap], a_dt)
                nc.sync.dma_start(at[:], AT[c])
                xts.append(xt)
                ats.append(at)

            def step1(c):
                """xws = (X*dis) @ W; two s-chunks per PSUM bank, one
                accumulation group (first matmul zeroes the bank)."""
                xt = xts[c]
                banks = []
                for h in range(hh):
                    ps = ps1_pool.tile([P, 2, f_out], f32)
                    for j in range(2):
                        s = 2 * h + j
                        for k in range(kc):
                            if c == 0:
                                lhs = xt[h][:, k, j * P:(j + 1) * P]
                            else:
                                lhs = xt[:, k, s * P:(s + 1) * P]
                            nc.tensor.matmul(
                                ps[:, j, :],
                                lhsT=lhs,
                                rhs=wts[k][:],
                                start=(j == 0 and k == 0),
                                stop=(j == 1 and k == kc - 1),
                                skip_group_check=True,
                            )
                    banks.append(ps)
                return banks

            def evac1(banks):
                """PSUM -> SBUF evacuation of step-1, split ACT/DVE."""
                his, los = [], []
                x_dt = fp8 if dr else fp16
                for h, ps in enumerate(banks):
                    hi = xw_pool.tile([P, 2, f_out], x_dt)
                    # DR: both hi on ACT (DVE owns the lo residuals);
                    # non-DR: one per engine
                    if dr or h == 0:
                        nc.scalar.activation(hi[:], ps[:], Copy)
                    else:
                        nc.vector.tensor_copy(hi[:], ps[:])
                    his.append(hi)
                    if dr:
                        lo = lo_pool.tile([P, 2, f_out], fp8)
                        nc.vector.scalar_tensor_tensor(
                            lo[:], ps[:], 1.0, hi[:], mul_op, sub_op)
                        los.append(lo)
                return his, los

            def step2(c, his, los):
                """Z_c^T[f, d] = sum_s xws[s, f] * At[s, d]; z f-halves
                evac'd and stored independently to keep the tail short."""
                at = ats[c]
                last = c == cpc - 1
                for f in range(fc):
                    ps = ps2_pool.tile([P, cap], f32)
                    if dr:
                        mms = [(his, 0), (his, 1), (los, 0), (los, 1)]
                        for i, (tiles, tp) in enumerate(mms):
                            nc.tensor.matmul(
                                ps[:],
                                lhsT=tiles[tp][:, :, f * P:(f + 1) * P],
                                rhs=at[:, 2 * tp:2 * tp + 2, :],
                                start=(i == 0),
                                stop=(i == len(mms) - 1),
                                perf_mode=DR,
                            )
                    else:
                        for s in range(sch):
                            nc.tensor.matmul(
                                ps[:],
                                lhsT=his[s // 2][:, s % 2, f * P:(f + 1) * P],
                                rhs=at[:, s, :],
                                start=(s == 0),
                                stop=(s == sch - 1),
                            )
                    ot = out_pool.tile([P, cap], fp16)
                    if last:
                        # final cluster: split the evac across both engines
                        # and both store queues to shorten the drain chain
                        nc.scalar.copy(ot[:, :cap // 2], ps[:, :cap // 2])
                        nc.vector.tensor_copy(
                            ot[:, cap // 2:], ps[:, cap // 2:])
                        q = nc.gpsimd if f == 0 else nc.sync
                        q.dma_start(YTr[c, :, f], ot[:])
                    else:
                        if f == 0:
                            nc.scalar.copy(ot[:], ps[:])
                        else:
                            nc.vector.tensor_copy(ot[:], ps[:])
                        nc.gpsimd.dma_start(YTr[c, :, f], ot[:])

            # software pipeline: step1(c+1) runs on the PE while the ACT/DVE
            # engines evacuate cluster c's xw banks ahead of step2(c); the
            # next cluster's evac is queued before this cluster's z copies
            # so the xw casts never arrive late on the engine queues
            banks = step1(0)
            his, los = evac1(banks)
            for c in range(cpc):
                if c + 1 < cpc:
                    nbanks = step1(c + 1)
                    nhis, nlos = evac1(nbanks)
                else:
                    nhis, nlos = None, None
                step2(c, his, los)
                his, los = nhis, nlos

    nc.compile()
    _prog_cache[key] = nc
    return nc


def _host_prep(X, W, b, assign, full_ei):
    """Shard + preprocess. Returns (in_maps, a_fp8, gather info)."""
    n, in_c = X.shape
    f_out = W.shape[1]
    src = full_ei[0].astype(np.int64)
    dst = full_ei[1].astype(np.int64)
    a_s = assign[src]
    intra = a_s == assign[dst]
    es, ed = src[intra], dst[intra]

    deg = np.ones(n, np.float32)
    np.add.at(deg, ed, np.float32(1))
    dis = (1.0 / np.sqrt(deg)).astype(np.float32)

    has_edge = np.zeros(N_CLUSTERS, bool)
    has_edge[np.unique(a_s[intra])] = True

    sizes = np.bincount(assign, minlength=N_CLUSTERS)
    cpc = -(-N_CLUSTERS // N_CORES)            # clusters per core
    cap = max(512, int(-(-sizes.max() // 256)) * 256)  # padded cluster size

    starts = np.zeros(N_CLUSTERS + 1, np.int64)
    starts[1:] = np.cumsum(sizes)
    order = np.argsort(assign, kind="stable")
    pos = np.empty(n, np.int64)
    pos[order] = np.arange(n) - starts[assign[order]]

    ctot = cpc * N_CORES
    # At blocks: At[c][s, d] = #edges(s->d) + [s==d]
    At = np.zeros((ctot, cap, cap), np.uint16)
    np.add.at(At, (assign[es], pos[es], pos[ed]), 1)
    At[assign, pos, pos] += 1
    a_fp8 = int(At.max()) <= 15    # integers <= 16 are exact in e4m3

    import ml_dtypes
    a_np = ml_dtypes.float8_e4m3 if a_fp8 else np.float16

    # pre-scale X rows by dis so the device never touches dis
    Xp = np.zeros((ctot, cap, in_c), np.float32)
    Xp[assign, pos] = X * dis[:, None]
    XT_all = np.ascontiguousarray(
        Xp.reshape(ctot * cap, in_c).T).astype(np.float16)

    # [c, s, d] -> [c, p, so, d] so each partition row is one contiguous run
    sch = cap // P
    At_send = np.ascontiguousarray(
        At.astype(a_np).reshape(-1, sch, P, cap).transpose(0, 2, 1, 3))

    nodes = cpc * cap
    in_maps = []
    for i in range(N_CORES):
        in_maps.append({
            "XT": np.ascontiguousarray(XT_all[:, i * nodes:(i + 1) * nodes]),
            "Wt": W.astype(np.float16),
            "AT": At_send[i * cpc:(i + 1) * cpc],
        })
    return in_maps, a_fp8, (cpc, cap, has_edge, pos, dis)


def _run(inputs, trace=False, tmpdir=None):
    from concourse.bass_utils import run_bass_kernel_spmd

    X = np.asarray(inputs["X"], np.float32)
    W = np.asarray(inputs["W"], np.float32)
    b = np.asarray(inputs["b"], np.float32)
    assign = np.asarray(inputs["assign"])
    full_ei = np.asarray(inputs["full_ei"])

    n, in_c = X.shape
    f_out = W.shape[1]
    in_maps, a_fp8, (cpc, cap, has_edge, pos, dis) = _host_prep(
        X, W, b, assign, full_ei)
    nc = _build_program(cpc, cap, in_c, f_out, a_fp8, _DR and a_fp8)

    res = run_bass_kernel_spmd(
        nc, in_maps, core_ids=list(range(N_CORES)),
        trace=trace, tmpdir=tmpdir,
    )
    # YT: [core][cpc, f_out, cap]; row n lives at [core, lc, :, pos]
    YTdev = np.stack([res.results[i]["YT"] for i in range(N_CORES)])
    if YTdev.dtype != np.float32:
        YTdev = YTdev.astype(np.float32)

    c = assign.astype(np.int64)
    core = c // cpc
    lc = c % cpc
    Y = YTdev[core, lc, :, pos]
    Y *= dis[:, None]
    Y += b[None, :].astype(np.float32)
    miss = ~has_edge[c]
    if miss.any():
        Y[miss] = X[miss]
    return Y, res


def kernel(**inputs) -> np.ndarray:
    Y, _ = _run(inputs)
    return Y
